# revision 4
# baseline (speedup 1.0000x reference)
"""Cost-volume kernel for Trainium2 (8 NeuronCores, data-parallel over B*H).

cost[b,h,w,d] = mean_c left[b,h,w,c] * right[b,h,w-(d+1),c], 0 where w-d-1 < 0
Shapes: B=4, H=256, W=512, C=64, D=64 (f32).

Strategy per core (128 independent (b,h) rows):
  - Host pre-transposes each core's inputs to [C, rows*W] bf16 (left also
    pre-scaled by 1/C), so device loads are plain contiguous DMAs (8KB per
    partition run) with half the bytes of the old f32 XBAR-transpose path.
  - TensorE banded matmuls, K=64: each 128-wide w block is two M=64
    matmuls packed into psum partition halves via tile_position col
    groups, giving a [128, 127] psum rect per block whose band (the D=64
    output diagonals, d' = 63-d) sits at cols (p mod 64) + d'.
  - DVE/ACT evict psum (f32 -> bf16) into a per-group SBUF rect
    (contiguous), and one full-rate DMA stores the rect as the kernel
    output.  There is NO on-device band extraction: the host unshard
    reads the band directly from the rect with an as_strided view (the
    +1-element-per-partition shear is just a stride there), flips
    d' -> d, and casts bf16 -> f32.
Per-core DMA traffic: 16.8MB loads + 16.6MB store, all >=4KB contiguous
runs (~93us at 360GB/s); measured ~109us/iter vs 252us for the prior
f32-bitcast + on-device-extract kernel.
"""

import numpy as np

N_CORES = 8
B_FULL, H_FULL, W, C = 4, 256, 512, 64
D = 64
ROWS = B_FULL * H_FULL           # 1024 independent rows
ROWS_PER_CORE = ROWS // N_CORES  # 128
RG = 8                           # rows per group
NBLK = W // 128                  # 128-wide w blocks per row
MSUB = 64                        # matmul M (psum col-group packing)
RECT2 = MSUB + 63                # psum rect cols per block


def build_nc(rows=ROWS_PER_CORE, l_eng="sync", r_eng="sync",
             store_eng="sync", ev="alt", lt_bufs=5, rect_bufs=4,
             ps_bufs=8, repeat=1, rg=RG, msub=MSUB):
    import concourse.mybir as mybir
    import concourse.tile as tile
    from concourse import bacc

    rect2 = msub + 63
    nsub = 128 // msub
    ng = rows // rg
    nblocks = rg * NBLK
    bcols = nblocks * rect2

    nc = bacc.Bacc()
    left = nc.declare_dram_parameter("left", [C, rows * W], mybir.dt.bfloat16,
                                     isOutput=False)
    right = nc.declare_dram_parameter("right", [C, rows * W],
                                      mybir.dt.bfloat16, isOutput=False)
    out = nc.declare_dram_parameter("out", [ng * 128, bcols],
                                    mybir.dt.bfloat16, isOutput=True)

    with tile.TileContext(nc) as tc:
        with (
            tc.tile_pool(name="lt", bufs=lt_bufs) as lt_pool,
            tc.tile_pool(name="rt", bufs=lt_bufs) as rt_pool,
            tc.tile_pool(name="rect", bufs=rect_bufs) as rect_pool,
            tc.tile_pool(name="ps", bufs=ps_bufs, space="PSUM") as psum_pool,
        ):
          for _rep in range(repeat):
            for g in range(ng):
                row0 = g * rg
                Lt = lt_pool.tile([C, rg * W], mybir.dt.bfloat16, tag="lt")
                Rt = rt_pool.tile([C, rg * W], mybir.dt.bfloat16, tag="rt")
                getattr(nc, l_eng).dma_start(
                    Lt[:, :], left[:, row0 * W:(row0 + rg) * W])
                getattr(nc, r_eng).dma_start(
                    Rt[:, :], right[:, row0 * W:(row0 + rg) * W])

                Brect = rect_pool.tile([128, bcols], mybir.dt.bfloat16,
                                       tag="rect")
                for r in range(rg):
                    for half in range(NBLK // 2):
                        i0, i1 = 2 * half, 2 * half + 1
                        col0 = (r * NBLK + i0) * rect2
                        P = psum_pool.tile([128, 2 * rect2], mybir.dt.float32,
                                           tag="ps")
                        for bi, i in enumerate((i0, i1)):
                            wl = i * 128
                            pc = bi * rect2
                            rb = r * W
                            for s in range(nsub):
                                ws = wl + s * msub
                                p0, p1 = s * msub, (s + 1) * msub
                                if i == 0 and ws < 64:
                                    # w' < 0 head undefined: zero it, compute
                                    # the valid tail starting at w' = 0
                                    zc = 64 - ws
                                    nc.vector.memset(P[p0:p1, pc: pc + zc],
                                                     0.0)
                                    nc.tensor.matmul(
                                        P[p0:p1, pc + zc: pc + rect2],
                                        Lt[:, rb + ws: rb + ws + msub],
                                        Rt[:, rb: rb + rect2 - zc],
                                        start=True, stop=True,
                                        tile_position=(0, p0))
                                else:
                                    nc.tensor.matmul(
                                        P[p0:p1, pc: pc + rect2],
                                        Lt[:, rb + ws: rb + ws + msub],
                                        Rt[:, rb + ws - 64:
                                           rb + ws + rect2 - 64],
                                        start=True, stop=True,
                                        tile_position=(0, p0))
                        ev_dst = Brect[:, col0: col0 + 2 * rect2]
                        if (ev == "alt" and half % 2 == 0) or ev == "vector":
                            nc.vector.tensor_copy(ev_dst, P[:, 0:2 * rect2])
                        else:
                            nc.scalar.copy(ev_dst, P[:, 0:2 * rect2])

                getattr(nc, store_eng).dma_start(
                    out[g * 128:(g + 1) * 128, :], Brect[:, :])

    nc.compile()
    return nc


def _prep_core(lf, rf):
    """[rows*W, C] f32 x2 -> transposed bf16 [C, rows*W] x2 (left / C)."""
    import ml_dtypes
    lt = np.ascontiguousarray(lf.T / C).astype(ml_dtypes.bfloat16)
    rt = np.ascontiguousarray(rf.T).astype(ml_dtypes.bfloat16)
    return lt, rt


def _unshard_core(flat, rows=ROWS_PER_CORE, msub=MSUB, rg=RG):
    """out flat bf16 [(rows//rg)*128*bcols] -> [rows, W, D] f32.

    Band element (g, h, q, r, ib, dp) lives at
      g*128*bcols + (msub*h + q)*bcols + (r*NBLK + ib)*rect2 + q + dp
    and maps to out[g*rg + r, 128*ib + msub*h + q, 63 - dp].
    """
    rect2 = msub + 63
    nsub = 128 // msub
    bcols = rg * NBLK * rect2
    ng = rows // rg
    st = flat.strides[0]
    v = np.lib.stride_tricks.as_strided(
        flat, shape=(ng, nsub, msub, rg, NBLK, 64),
        strides=(128 * bcols * st, msub * bcols * st, (bcols + 1) * st,
                 NBLK * rect2 * st, rect2 * st, st))
    v = v.transpose(0, 3, 4, 1, 2, 5)      # g, r, ib, h, q, dp
    v = v.reshape(rows, W, D).astype(np.float32)
    return v[:, :, ::-1]


_NC_CACHE = {}


def kernel(left_feature, right_feature):
    from concourse.bass_utils import run_bass_kernel_spmd

    lf = np.ascontiguousarray(left_feature, dtype=np.float32).reshape(
        ROWS, W, C)
    rf = np.ascontiguousarray(right_feature, dtype=np.float32).reshape(
        ROWS, W, C)

    if "nc" not in _NC_CACHE:
        _NC_CACHE["nc"] = build_nc()
    nc = _NC_CACHE["nc"]

    in_maps = []
    for k in range(N_CORES):
        sl = slice(k * ROWS_PER_CORE, (k + 1) * ROWS_PER_CORE)
        lt, rt = _prep_core(lf[sl].reshape(ROWS_PER_CORE * W, C),
                            rf[sl].reshape(ROWS_PER_CORE * W, C))
        in_maps.append({"left": lt, "right": rt})

    res = run_bass_kernel_spmd(nc, in_maps, core_ids=list(range(N_CORES)))

    out = np.empty((ROWS, W, D), dtype=np.float32)
    for k in range(N_CORES):
        flat = np.asarray(res.results[k]["out"]).reshape(-1)
        out[k * ROWS_PER_CORE:(k + 1) * ROWS_PER_CORE] = _unshard_core(flat)
    return out.reshape(B_FULL, H_FULL, W, D)
